# revision 25
# baseline (speedup 1.0000x reference)
"""Multi-head attention (query-axis softmax variant) on 8 Trainium2 NeuronCores.

Problem: B=4, T=2048, C=1024, H=16, Dh=64.
  q/k/v = per-head projections of x; wei = (q k^T) * C**-0.5, causal-masked;
  softmax over the QUERY axis (axis=2 of (B,H,T,S)); out = attn @ v, concat
  heads, project with Wp and add bp.

Sharding: 8 cores = 4 batches x 2 head-groups (8 heads each).  Each core
computes a partial projection output for its batch; host sums the two
group partials per batch and adds the bias.

Per-core dataflow is fully "transposed" (features on partitions, tokens on
the free axis) so the query-axis softmax stats become free-axis reductions:
  xT (C, T) -> qT/kT per head-pair (128, T) -> scores W[s,t] per key-tile
  -> P = exp(W*scale) with the masked entries driven to 0 via a -1e30
  additive triangle, Z[s] = accumulated row sums from the Exp activation
  -> v' = v * (1/Z) -> attout^T[d,t] -> y = attout^T.T @ WpT.

QKV projections run in fp32r (full-rate at K=128); attention and the output
projection run in bf16 (P in [0, ~1.1], v'/ao cast at PSUM evacuation).
The two heads of a pair are interleaved so their K=64 score matmuls land in
different PE row groups (and their M=64 attout matmuls in different column
groups) and execute concurrently; the next pair's q/k projection matmuls are
interleaved into the ACT-bound attention phase to keep the PE busy.
"""
import numpy as np

T = 2048
C = 1024
H = 16
DH = 64
B = 4
SCALE = float(C) ** -0.5
NEG = -1e30
P = 128

_CACHE = {}


def _build_nc():
    import concourse.bacc as bacc
    import concourse.tile as tile
    import concourse.mybir as mybir
    from contextlib import ExitStack

    FP = mybir.dt.float32
    FR = mybir.dt.float32r
    BF = mybir.dt.bfloat16
    AX = mybir.AxisListType.X
    EXP = mybir.ActivationFunctionType.Exp

    nc = bacc.Bacc("TRN2", target_bir_lowering=False, debug=False, num_devices=8)

    xT_d = nc.declare_dram_parameter("xt", [C, T], FR, isOutput=False)
    wq_d = nc.declare_dram_parameter("wq", [C, 512], FR, isOutput=False)
    wk_d = nc.declare_dram_parameter("wk", [C, 512], FR, isOutput=False)
    wv_d = nc.declare_dram_parameter("wv", [C, 512], FR, isOutput=False)
    wp_d = nc.declare_dram_parameter("wpt", [512, C], BF, isOutput=False)
    tri_d = nc.declare_dram_parameter("tri", [P, P], FP, isOutput=False)
    y_d = nc.declare_dram_parameter("y", [T, C], FP, isOutput=True)

    NCT = C // P      # 8 c-tiles
    NST = T // P      # 16 s-tiles
    NTG = T // 512    # 4 t-groups

    with tile.TileContext(nc) as tc:
        with (
            tc.tile_pool(name="perm", bufs=1) as perm,
            tc.tile_pool(name="work", bufs=4) as work,
            tc.tile_pool(name="stat", bufs=3) as stat,
            tc.tile_pool(name="statv", bufs=4) as statv,
            tc.tile_pool(name="pao", bufs=1) as pao,
            tc.tile_pool(name="ps", bufs=2, space="PSUM") as pspool,
            tc.tile_pool(name="avps", bufs=4, space="PSUM") as avpool,
        ):
            tri = perm.tile([P, P], FP, tag="tri")
            nc.sync.dma_start(tri[:], tri_d[:])
            v_sb = perm.tile([P, NST, 512], BF, tag="v")
            q_sb = perm.tile([P, 4, T], BF, tag="q")
            k_sb = perm.tile([P, 4, T], BF, tag="k")

            es = ExitStack()
            pxw = es.enter_context(tc.tile_pool(name="px", bufs=1))
            wpool = es.enter_context(tc.tile_pool(name="w", bufs=2))

            xT = pxw.tile([P, NCT, T], FR, tag="xT")
            nc.sync.dma_start(xT[:], xT_d.ap().rearrange("(a c) t -> c a t", c=P))
            wv = pxw.tile([P, NCT, 512], FR, tag="wv")
            nc.sync.dma_start(wv[:], wv_d.ap().rearrange("(a c) m -> c a m", c=P))

            # Warm up the PE's HAM clock gate while the big input DMAs land:
            # ~10us of junk matmuls so the real work starts at 2.4 GHz.
            warm = perm.tile([P, 512], BF, tag="warm")
            nc.vector.memset(warm[:], 0.0)
            for wi in range(24):
                wps = pspool.tile([P, 1024], FP, tag="ps")
                for _ in range(2):
                    nc.tensor.matmul(wps[:, :512], lhsT=warm[:, :P], rhs=warm[:],
                                     start=True, stop=True)

            # ---- phase 1: qT/kT per pair (128 = [h0 d, h1 d], T), bf16 out ----
            def emit_qk_steps(p):
                """Returns a list of closures; each emits one 512-col psum group."""
                wqt = wpool.tile([P, NCT, P], FR, tag="wq")
                wkt = wpool.tile([P, NCT, P], FR, tag="wk")
                nc.sync.dma_start(
                    wqt[:], wq_d.ap()[:, P * p:P * p + P].rearrange("(a c) m -> c a m", c=P))
                nc.sync.dma_start(
                    wkt[:], wk_d.ap()[:, P * p:P * p + P].rearrange("(a c) m -> c a m", c=P))
                steps = []
                for wt, dst in ((wqt, q_sb), (wkt, k_sb)):
                    for g in range(NTG):
                        def step(wt=wt, dst=dst, g=g, p=p, pool=None):
                            pl = pool if pool is not None else pspool
                            psq = pl.tile([P, 512], FP, tag="av" if pool is not None else "ps",
                                          name=f"qkps{p}_{g}_{dst is k_sb}")
                            for ct in range(NCT):
                                nc.tensor.matmul(
                                    psq[:, :512], lhsT=wt[:, ct, :],
                                    rhs=xT[:, ct, 512 * g:512 * g + 512],
                                    start=(ct == 0), stop=(ct == NCT - 1))
                            nc.vector.tensor_copy(dst[:, p, 512 * g:512 * g + 512], psq[:, :512])
                        steps.append(step)
                return steps

            def v_step(st, pool=None):
                pl = pool if pool is not None else pspool
                ps = pl.tile([P, 512] if pool is not None else [P, 1024], FP,
                             tag="av" if pool is not None else "ps", name=f"vps{st}")
                for ct in range(NCT):
                    nc.tensor.matmul(
                        ps[:, :512],
                        lhsT=xT[:, ct, P * st:P * st + P],
                        rhs=wv[:, ct, :],
                        start=(ct == 0), stop=(ct == NCT - 1))
                nc.vector.tensor_copy(v_sb[:, st, :], ps[:, :512])

            # ---- serial prefix: qk for pair 0 and the first half of v ----
            for step in emit_qk_steps(0):
                step()
            for st in range(8):
                v_step(st)

            # ---- phase 2: attention; the two heads of a pair run in lockstep
            # (score matmuls in different PE row groups, attout matmuls in
            # different column groups), and the attout matmuls of iteration
            # i-1 are emitted after the score matmuls of iteration i so the
            # in-order PE queue never stalls on the Exp results.
            ao = pao.tile([P, 4, T], BF, tag="ao")

            def emit_scores(p, i):
                """PE: both heads' score matmuls (chunk-interleaved so the
                row-group pair runs concurrently); DVE: diag masks."""
                t0 = P * i
                blocks = [(t0, 1024), (1024, 2048)] if i < 8 else [(t0, 2048)]
                prows = [work.tile([P, T], BF, tag="prow", bufs=8,
                                   name=f"prow{p}_{i}_{h}") for h in range(2)]
                zps = [stat.tile([P, 2], FP, tag="zp", bufs=8,
                                 name=f"zp{p}_{i}_{h}") for h in range(2)]
                tiles = []
                for bi, (lo, hi) in enumerate(blocks):
                    sps2 = [pspool.tile([P, 1024], FP, tag="ps",
                                        name=f"sps{p}_{i}_{bi}_{h}") for h in range(2)]
                    for clo in range(lo, hi, 512):
                        chi = min(clo + 512, hi)
                        for hl in range(2):
                            hb = 64 * hl
                            nc.tensor.matmul(
                                sps2[hl][:, clo - lo:chi - lo],
                                lhsT=k_sb[hb:hb + 64, p, t0:t0 + P],
                                rhs=q_sb[hb:hb + 64, p, clo:chi],
                                start=True, stop=True)
                    if lo == t0:
                        for hl in range(2):
                            nc.vector.tensor_add(sps2[hl][:, 0:P], sps2[hl][:, 0:P], tri[:])
                    tiles.append((sps2, bi, lo, hi))
                return dict(i=i, t0=t0, nb=len(blocks), prows=prows, zps=zps, tiles=tiles)

            def emit_exps(sc):
                for (sps2, bi, lo, hi) in sc["tiles"]:
                    for hl in range(2):
                        nc.scalar.activation(
                            sc["prows"][hl][:, lo:hi], sps2[hl][:, :hi - lo], EXP,
                            scale=SCALE, accum_out=sc["zps"][hl][:, bi:bi + 1])

            def emit_stats(p, sc):
                vps = []
                for hl in range(2):
                    z = stat.tile([P, 1], FP, tag="z", name=f"z{p}_{sc['i']}_{hl}")
                    if sc["nb"] == 2:
                        nc.gpsimd.tensor_add(z[:], sc["zps"][hl][:, 0:1], sc["zps"][hl][:, 1:2])
                    else:
                        nc.gpsimd.tensor_copy(z[:], sc["zps"][hl][:, 0:1])
                    rz = stat.tile([P, 1], FP, tag="rz", name=f"rz{p}_{sc['i']}_{hl}")
                    nc.vector.reciprocal(rz[:], z[:])
                    vp = statv.tile([P, 64], BF, tag="vp", bufs=6, name=f"vp{p}_{sc['i']}_{hl}")
                    hh = 64 * (2 * p + hl)
                    nc.gpsimd.tensor_scalar_mul(vp[:], v_sb[:, sc["i"], hh:hh + 64], rz[:])
                    vps.append(vp)
                return vps

            wpt = pao.tile([P, 4, C], BF, tag="wpt")
            nc.sync.dma_start(wpt[:], wp_d.ap().rearrange("(a c) m -> c a m", c=P))

            def proj_group(tt, nb):
                ps = avpool.tile([P, 512], FP, tag="av", name=f"pps{tt}_{nb}")
                for pp in range(4):
                    nc.tensor.matmul(
                        ps[:, :512], lhsT=ao[:, pp, P * tt:P * tt + P],
                        rhs=wpt[:, pp, 512 * nb:512 * nb + 512],
                        start=(pp == 0), stop=(pp == 3))
                yt = work.tile([P, 512], FP, tag="yt", bufs=2, name=f"yt{tt}_{nb}")
                nc.vector.tensor_copy(yt[:], ps[:, :512])
                nc.sync.dma_start(y_d.ap()[P * tt:P * tt + P, 512 * nb:512 * nb + 512], yt[:])

            for p in range(4):
                if p == 0:
                    fill = [(lambda st=st: (lambda pool=None: v_step(st, pool=pool)))()
                            for st in range(8, NST)] + emit_qk_steps(1)
                elif p < 3:
                    fill = emit_qk_steps(p + 1)
                else:
                    fill = [(lambda tt=tt, nb=nb: (lambda pool=None: proj_group(tt, nb)))()
                            for tt in range(4) for nb in range(2)]
                avc = [avpool.tile([P, 512], FP, tag="av", name=f"avc{p}_{c}")
                       for c in range(NTG)]
                pend = []
                done_av = -1
                evacd = 0
                sc = emit_scores(p, 0)
                for i in range(NST):
                    sc_next = emit_scores(p, i + 1) if i < NST - 1 else None
                    emit_exps(sc)
                    if len(pend) >= 2 or (i == NST - 1 and pend):
                        pend.pop(0)()
                        done_av += 1
                    # evacuate finished attout chunks (frees their psum bank
                    # for the qk / projection filler steps)
                    if evacd < 3 and done_av == 4 * evacd + 3:
                        nc.vector.tensor_copy(
                            ao[:, p, 512 * evacd:512 * evacd + 512], avc[evacd][:])
                        evacd += 1
                    if fill and i >= 6:
                        n = 2 if len(fill) > (NST - 1 - i) else 1
                        for _ in range(min(n, len(fill))):
                            fill.pop(0)(pool=avpool)
                    vps = emit_stats(p, sc)

                    def make_av(i=i, t0=P * i, vps=vps, prows=sc["prows"]):
                        def emit():
                            for c in range(NTG):
                                clo, chi = 512 * c, 512 * c + 512
                                lo2 = max(clo, t0)
                                if lo2 >= chi:
                                    continue
                                for hl in range(2):
                                    hb = 64 * hl
                                    nc.tensor.matmul(
                                        avc[c][hb:hb + 64, lo2 - clo:512],
                                        lhsT=vps[hl][:], rhs=prows[hl][:, lo2:chi],
                                        start=(i == 0), stop=(i == 4 * c + 3))
                        return emit

                    pend.append(make_av())
                    sc = sc_next
                for e in pend:
                    e()
                for step in fill:
                    step(pool=avpool)
                nc.vector.tensor_copy(ao[:, p, 512 * 3:], avc[3][:])
            es.close()

            # ---- phase 3: projection y[t, c'] ----
            for tt in range(4, NST):
                for nb in range(2):
                    proj_group(tt, nb)

    nc.compile()
    return nc


def _get_nc():
    if "nc" not in _CACHE:
        _CACHE["nc"] = _build_nc()
    return _CACHE["nc"]


def _in_maps(x, Wq, Wk, Wv, Wp):
    import ml_dtypes
    tri = np.tril(np.full((P, P), NEG, np.float32), -1)
    maps = []
    for b in range(B):
        xT = np.ascontiguousarray(x[b].T)
        for g in range(2):
            heads = range(8 * g, 8 * g + 8)
            maps.append({
                "xt": xT,
                "wq": np.ascontiguousarray(np.concatenate([Wq[h] for h in heads], 1)),
                "wk": np.ascontiguousarray(np.concatenate([Wk[h] for h in heads], 1)),
                "wv": np.ascontiguousarray(np.concatenate([Wv[h] for h in heads], 1)),
                "wpt": np.ascontiguousarray(Wp[:, 512 * g:512 * g + 512].T).astype(ml_dtypes.bfloat16),
                "tri": tri,
            })
    return maps


def kernel(x, Wq, Wk, Wv, Wp, bp):
    from concourse.bass_utils import run_bass_kernel_spmd

    x = np.asarray(x, np.float32)
    Wq = np.asarray(Wq, np.float32)
    Wk = np.asarray(Wk, np.float32)
    Wv = np.asarray(Wv, np.float32)
    Wp = np.asarray(Wp, np.float32)
    bp = np.asarray(bp, np.float32)

    nc = _get_nc()
    res = run_bass_kernel_spmd(nc, _in_maps(x, Wq, Wk, Wv, Wp), list(range(8)))
    y = np.empty((B, T, C), np.float32)
    for b in range(B):
        y[b] = res.results[2 * b]["y"] + res.results[2 * b + 1]["y"] + bp
    return y


# revision 26
# speedup vs baseline: 1.0889x; 1.0889x over previous
"""Multi-head attention (query-axis softmax variant) on 8 Trainium2 NeuronCores.

Problem: B=4, T=2048, C=1024, H=16, Dh=64.
  q/k/v = per-head projections of x; wei = (q k^T) * C**-0.5, causal-masked;
  softmax over the QUERY axis (axis=2 of (B,H,T,S)); out = attn @ v, concat
  heads, project with Wp and add bp.

Sharding: 8 cores = 4 batches x 2 head-groups (8 heads each).  Each core
computes a partial projection output for its batch; host sums the two
group partials per batch and adds the bias.

Per-core dataflow is fully "transposed" (features on partitions, tokens on
the free axis) so the query-axis softmax stats become free-axis reductions:
  xT (C, T) -> qT/kT per head-pair (128, T) -> scores W[s,t] per key-tile
  -> P = exp(W*scale) with the masked entries driven to 0 via a -1e30
  additive triangle, Z[s] = accumulated row sums from the Exp activation
  -> v' = v * (1/Z) -> attout^T[d,t] -> y = attout^T.T @ WpT.

QKV projections run in fp32r (full-rate at K=128); attention and the output
projection run in bf16 (P in [0, ~1.1], v'/ao cast at PSUM evacuation).
The two heads of a pair are interleaved so their K=64 score matmuls land in
different PE row groups (and their M=64 attout matmuls in different column
groups) and execute concurrently; the next pair's q/k projection matmuls are
interleaved into the ACT-bound attention phase to keep the PE busy.
"""
import numpy as np

T = 2048
C = 1024
H = 16
DH = 64
B = 4
SCALE = float(C) ** -0.5
NEG = -1e30
P = 128

_CACHE = {}


def _build_nc():
    import concourse.bacc as bacc
    import concourse.tile as tile
    import concourse.mybir as mybir
    from contextlib import ExitStack

    FP = mybir.dt.float32
    FR = mybir.dt.float32r
    BF = mybir.dt.bfloat16
    AX = mybir.AxisListType.X
    EXP = mybir.ActivationFunctionType.Exp

    nc = bacc.Bacc("TRN2", target_bir_lowering=False, debug=False, num_devices=8)

    xT_d = nc.declare_dram_parameter("xt", [C, T], FR, isOutput=False)
    wq_d = nc.declare_dram_parameter("wq", [C, 512], FR, isOutput=False)
    wk_d = nc.declare_dram_parameter("wk", [C, 512], FR, isOutput=False)
    wv_d = nc.declare_dram_parameter("wv", [C, 512], FR, isOutput=False)
    wp_d = nc.declare_dram_parameter("wpt", [512, C], BF, isOutput=False)
    tri_d = nc.declare_dram_parameter("tri", [P, P], FP, isOutput=False)
    y_d = nc.declare_dram_parameter("y", [T, C], FP, isOutput=True)

    NCT = C // P      # 8 c-tiles
    NST = T // P      # 16 s-tiles
    NTG = T // 512    # 4 t-groups

    with tile.TileContext(nc) as tc:
        with (
            tc.tile_pool(name="perm", bufs=1) as perm,
            tc.tile_pool(name="work", bufs=4) as work,
            tc.tile_pool(name="stat", bufs=3) as stat,
            tc.tile_pool(name="statv", bufs=4) as statv,
            tc.tile_pool(name="pao", bufs=1) as pao,
            tc.tile_pool(name="ps", bufs=2, space="PSUM") as pspool,
            tc.tile_pool(name="avps", bufs=4, space="PSUM") as avpool,
        ):
            tri = perm.tile([P, P], FP, tag="tri")
            nc.sync.dma_start(tri[:], tri_d[:])
            v_sb = perm.tile([P, NST, 512], BF, tag="v")
            q_sb = perm.tile([P, 4, T], BF, tag="q")
            k_sb = perm.tile([P, 4, T], BF, tag="k")

            es = ExitStack()
            pxw = es.enter_context(tc.tile_pool(name="px", bufs=1))
            wpool = es.enter_context(tc.tile_pool(name="w", bufs=2))

            xT = pxw.tile([P, NCT, T], FR, tag="xT")
            nc.sync.dma_start(xT[:], xT_d.ap().rearrange("(a c) t -> c a t", c=P))
            wv = pxw.tile([P, NCT, 512], FR, tag="wv")
            nc.sync.dma_start(wv[:], wv_d.ap().rearrange("(a c) m -> c a m", c=P))

            # Warm up the PE's HAM clock gate while the big input DMAs land:
            # ~10us of junk matmuls so the real work starts at 2.4 GHz.
            warm = perm.tile([P, 512], BF, tag="warm")
            nc.vector.memset(warm[:], 0.0)
            for wi in range(24):
                wps = pspool.tile([P, 1024], FP, tag="ps")
                for _ in range(2):
                    nc.tensor.matmul(wps[:, :512], lhsT=warm[:, :P], rhs=warm[:],
                                     start=True, stop=True)

            # ---- phase 1: qT/kT per pair (128 = [h0 d, h1 d], T), bf16 out ----
            def emit_qk_steps(p):
                """Returns a list of closures; each emits one 512-col psum group."""
                wqt = wpool.tile([P, NCT, P], FR, tag="wq")
                wkt = wpool.tile([P, NCT, P], FR, tag="wk")
                nc.sync.dma_start(
                    wqt[:], wq_d.ap()[:, P * p:P * p + P].rearrange("(a c) m -> c a m", c=P))
                nc.sync.dma_start(
                    wkt[:], wk_d.ap()[:, P * p:P * p + P].rearrange("(a c) m -> c a m", c=P))
                steps = []
                for wt, dst in ((wqt, q_sb), (wkt, k_sb)):
                    for g in range(NTG):
                        def step(wt=wt, dst=dst, g=g, p=p, pool=None):
                            pl = pool if pool is not None else pspool
                            psq = pl.tile([P, 512], FP, tag="av" if pool is not None else "ps",
                                          name=f"qkps{p}_{g}_{dst is k_sb}")
                            for ct in range(NCT):
                                nc.tensor.matmul(
                                    psq[:, :512], lhsT=wt[:, ct, :],
                                    rhs=xT[:, ct, 512 * g:512 * g + 512],
                                    start=(ct == 0), stop=(ct == NCT - 1))
                            nc.vector.tensor_copy(dst[:, p, 512 * g:512 * g + 512], psq[:, :512])
                        steps.append(step)
                return steps

            def v_step(st, pool=None):
                pl = pool if pool is not None else pspool
                ps = pl.tile([P, 512] if pool is not None else [P, 1024], FP,
                             tag="av" if pool is not None else "ps", name=f"vps{st}")
                for ct in range(NCT):
                    nc.tensor.matmul(
                        ps[:, :512],
                        lhsT=xT[:, ct, P * st:P * st + P],
                        rhs=wv[:, ct, :],
                        start=(ct == 0), stop=(ct == NCT - 1))
                nc.vector.tensor_copy(v_sb[:, st, :], ps[:, :512])

            # ---- serial prefix: qk for pair 0 and the first half of v ----
            for step in emit_qk_steps(0):
                step()
            for st in range(8):
                v_step(st)

            # ---- phase 2: attention; the two heads of a pair run in lockstep
            # (score matmuls in different PE row groups, attout matmuls in
            # different column groups), and the attout matmuls of iteration
            # i-1 are emitted after the score matmuls of iteration i so the
            # in-order PE queue never stalls on the Exp results.
            ao = pao.tile([P, 4, T], BF, tag="ao")

            def emit_scores(p, i):
                """PE: both heads' score matmuls (chunk-interleaved so the
                row-group pair runs concurrently); DVE: diag masks."""
                t0 = P * i
                blocks = [(t0, 1024), (1024, 2048)] if i < 8 else [(t0, 2048)]
                prows = [work.tile([P, T], BF, tag="prow", bufs=8,
                                   name=f"prow{p}_{i}_{h}") for h in range(2)]
                zps = [stat.tile([P, 2], FP, tag="zp", bufs=8,
                                 name=f"zp{p}_{i}_{h}") for h in range(2)]
                tiles = []
                for bi, (lo, hi) in enumerate(blocks):
                    sps2 = [pspool.tile([P, 1024], FP, tag="ps",
                                        name=f"sps{p}_{i}_{bi}_{h}") for h in range(2)]
                    for clo in range(lo, hi, 512):
                        chi = min(clo + 512, hi)
                        for hl in range(2):
                            hb = 64 * hl
                            nc.tensor.matmul(
                                sps2[hl][:, clo - lo:chi - lo],
                                lhsT=k_sb[hb:hb + 64, p, t0:t0 + P],
                                rhs=q_sb[hb:hb + 64, p, clo:chi],
                                start=True, stop=True)
                    if lo == t0:
                        for hl in range(2):
                            nc.vector.tensor_add(sps2[hl][:, 0:P], sps2[hl][:, 0:P], tri[:])
                    tiles.append((sps2, bi, lo, hi))
                return dict(i=i, t0=t0, nb=len(blocks), prows=prows, zps=zps, tiles=tiles)

            def emit_exps(sc):
                for (sps2, bi, lo, hi) in sc["tiles"]:
                    for hl in range(2):
                        nc.scalar.activation(
                            sc["prows"][hl][:, lo:hi], sps2[hl][:, :hi - lo], EXP,
                            scale=SCALE, accum_out=sc["zps"][hl][:, bi:bi + 1])

            def emit_stats(p, sc):
                vps = []
                for hl in range(2):
                    z = stat.tile([P, 1], FP, tag="z", name=f"z{p}_{sc['i']}_{hl}")
                    if sc["nb"] == 2:
                        nc.gpsimd.tensor_add(z[:], sc["zps"][hl][:, 0:1], sc["zps"][hl][:, 1:2])
                    else:
                        nc.gpsimd.tensor_copy(z[:], sc["zps"][hl][:, 0:1])
                    rz = stat.tile([P, 1], FP, tag="rz", name=f"rz{p}_{sc['i']}_{hl}")
                    nc.vector.reciprocal(rz[:], z[:])
                    vp = statv.tile([P, 64], BF, tag="vp", bufs=6, name=f"vp{p}_{sc['i']}_{hl}")
                    hh = 64 * (2 * p + hl)
                    nc.vector.tensor_scalar_mul(vp[:], v_sb[:, sc["i"], hh:hh + 64], rz[:])
                    vps.append(vp)
                return vps

            wpt = pao.tile([P, 4, C], BF, tag="wpt")
            nc.sync.dma_start(wpt[:], wp_d.ap().rearrange("(a c) m -> c a m", c=P))

            def proj_group(tt, nb):
                ps = avpool.tile([P, 512], FP, tag="av", name=f"pps{tt}_{nb}")
                for pp in range(4):
                    nc.tensor.matmul(
                        ps[:, :512], lhsT=ao[:, pp, P * tt:P * tt + P],
                        rhs=wpt[:, pp, 512 * nb:512 * nb + 512],
                        start=(pp == 0), stop=(pp == 3))
                yt = work.tile([P, 512], FP, tag="yt", bufs=2, name=f"yt{tt}_{nb}")
                nc.vector.tensor_copy(yt[:], ps[:, :512])
                nc.sync.dma_start(y_d.ap()[P * tt:P * tt + P, 512 * nb:512 * nb + 512], yt[:])

            for p in range(4):
                if p == 0:
                    fill = [(lambda st=st: (lambda pool=None: v_step(st, pool=pool)))()
                            for st in range(8, NST)] + emit_qk_steps(1)
                elif p < 3:
                    fill = emit_qk_steps(p + 1)
                else:
                    fill = [(lambda tt=tt, nb=nb: (lambda pool=None: proj_group(tt, nb)))()
                            for tt in range(4) for nb in range(2)]
                avc = [avpool.tile([P, 512], FP, tag="av", name=f"avc{p}_{c}")
                       for c in range(NTG)]
                pend = []
                done_av = -1
                evacd = 0
                sc = emit_scores(p, 0)
                for i in range(NST):
                    sc_next = emit_scores(p, i + 1) if i < NST - 1 else None
                    emit_exps(sc)
                    if len(pend) >= 2 or (i == NST - 1 and pend):
                        pend.pop(0)()
                        done_av += 1
                    # evacuate finished attout chunks (frees their psum bank
                    # for the qk / projection filler steps)
                    if evacd < 3 and done_av == 4 * evacd + 3:
                        nc.vector.tensor_copy(
                            ao[:, p, 512 * evacd:512 * evacd + 512], avc[evacd][:])
                        evacd += 1
                    if fill and i >= 6:
                        n = 2 if len(fill) > (NST - 1 - i) else 1
                        for _ in range(min(n, len(fill))):
                            fill.pop(0)(pool=avpool)
                    vps = emit_stats(p, sc)

                    def make_av(i=i, t0=P * i, vps=vps, prows=sc["prows"]):
                        def emit():
                            for c in range(NTG):
                                clo, chi = 512 * c, 512 * c + 512
                                lo2 = max(clo, t0)
                                if lo2 >= chi:
                                    continue
                                for hl in range(2):
                                    hb = 64 * hl
                                    nc.tensor.matmul(
                                        avc[c][hb:hb + 64, lo2 - clo:512],
                                        lhsT=vps[hl][:], rhs=prows[hl][:, lo2:chi],
                                        start=(i == 0), stop=(i == 4 * c + 3))
                        return emit

                    pend.append(make_av())
                    sc = sc_next
                for e in pend:
                    e()
                for step in fill:
                    step(pool=avpool)
                nc.vector.tensor_copy(ao[:, p, 512 * 3:], avc[3][:])
            es.close()

            # ---- phase 3: projection y[t, c'] ----
            for tt in range(4, NST):
                for nb in range(2):
                    proj_group(tt, nb)

    nc.compile()
    return nc


def _get_nc():
    if "nc" not in _CACHE:
        _CACHE["nc"] = _build_nc()
    return _CACHE["nc"]


def _in_maps(x, Wq, Wk, Wv, Wp):
    import ml_dtypes
    tri = np.tril(np.full((P, P), NEG, np.float32), -1)
    maps = []
    for b in range(B):
        xT = np.ascontiguousarray(x[b].T)
        for g in range(2):
            heads = range(8 * g, 8 * g + 8)
            maps.append({
                "xt": xT,
                "wq": np.ascontiguousarray(np.concatenate([Wq[h] for h in heads], 1)),
                "wk": np.ascontiguousarray(np.concatenate([Wk[h] for h in heads], 1)),
                "wv": np.ascontiguousarray(np.concatenate([Wv[h] for h in heads], 1)),
                "wpt": np.ascontiguousarray(Wp[:, 512 * g:512 * g + 512].T).astype(ml_dtypes.bfloat16),
                "tri": tri,
            })
    return maps


def kernel(x, Wq, Wk, Wv, Wp, bp):
    from concourse.bass_utils import run_bass_kernel_spmd

    x = np.asarray(x, np.float32)
    Wq = np.asarray(Wq, np.float32)
    Wk = np.asarray(Wk, np.float32)
    Wv = np.asarray(Wv, np.float32)
    Wp = np.asarray(Wp, np.float32)
    bp = np.asarray(bp, np.float32)

    nc = _get_nc()
    res = run_bass_kernel_spmd(nc, _in_maps(x, Wq, Wk, Wv, Wp), list(range(8)))
    y = np.empty((B, T, C), np.float32)
    for b in range(B):
        y[b] = res.results[2 * b]["y"] + res.results[2 * b + 1]["y"] + bp
    return y
